# revision 29
# baseline (speedup 1.0000x reference)
"""CAM (channel attention) module kernel for Trainium2, SPMD over 8 NeuronCores.

Reference computation (per batch b):
    q = x[b].reshape(C, N)                  # C=64, N=H*W=65536
    energy = q @ q.T                        # [C, C]
    att = softmax(rowmax(energy) - energy)  # == softmax(-energy) rows
    out[b] = gamma * (att @ q) + x[b]

Sharding: data-parallel over batch, 2 batches per core, no cross-core comm.

Per-core design (v2, bf16 hi/lo split for PE speed, fp32-grade accuracy):

  Layout: q2 [128, 32768] fp32 where partition p = h*64 + c (h = n-half,
  c = channel), streamed in [128, 2048] tiles (two [64, 2048] DMAs each so
  the HWDGE spreads descriptors over all 16 SDMA engines).

  Split: hi = bf16(q), lo = bf16(q - hi) on GpSimd (idle engine), after
  which the fp32 tile is released (hi+lo reconstructs q to ~2^-18).

  Phase 1 (energy): PE-transpose [128, 128] bf16 blocks of hi/lo (one op
  covers both n-halves), stage groups of 8 in a PSUM bank, DVE-copy to
  SBUF, then bf16 pair-gram matmuls accumulate
      G  += Thi^T @ Thi   (diag blocks = per-half energies)
      X  += Thi^T @ Tlo   (hi-lo cross term; lo.lo term is negligible)
  E = (G00+G11) + (X00+X11) + (X00+X11)^T, where the half sums are done
  with one matmul against the stacked double identity.

  Softmax: att = exp(rmin(E) - E) / rowsum (shift-invariant == reference).
  Build S = blockdiag(M^T, M^T), M = gamma*att + I, split S into bf16
  hi/lo. The identity on the diagonal carries the residual.

  Phase 2: out_slab = S_hi@hi + S_hi@lo + S_lo@hi (3 bf16 matmuls into one
  PSUM bank), DVE copy to staging, two [64, 2048] DMAs out per tile.
"""

import os

import numpy as np

import concourse.bass as bass
import concourse.tile as tile
from concourse import bacc, mybir

# Problem constants (hardcoded per harness contract).
B, C, H, W = 16, 64, 256, 256
N = H * W  # 65536
NCORES = 8
BPC = B // NCORES  # batches per core

# Tunables.
TILE_F = 2048  # free width of a q2 tile
CHUNK = 128  # n'-block width (covers both halves per transpose/gram)
TGROUP = 8  # transposed blocks per PSUM bank group
SLAB = 512  # phase-2 moving width
USE_LO_ENERGY = os.environ.get("CAM_LO_ENERGY", "1") == "1"
USE_LO_PHASE2 = os.environ.get("CAM_LO_PHASE2", "1") == "1"


def build_cam_program(n=N, bpc=BPC, tile_f=TILE_F):
    """Build the single-core Bass program (same program runs on all cores)."""
    half = n // 2
    ntiles = half // tile_f
    fp32 = mybir.dt.float32
    bf16 = mybir.dt.bfloat16

    # Bacc (not plain Bass): its finalize() runs move_matmul_waits_to_ldweights
    # and generate_event_semaphores, which split multi-sem waits down to the
    # TRN2 limit of one embedded wait per instruction.
    nc = bacc.Bacc("TRN2", target_bir_lowering=False, debug=False)
    x = nc.dram_tensor("x", [bpc, C, n], fp32, kind="ExternalInput").ap()
    gamma = nc.dram_tensor("gamma", [1], fp32, kind="ExternalInput").ap()
    # ident2: [128, 64] stacked double identity (fp32) for half-sum matmuls.
    ident = nc.dram_tensor("ident", [128, 64], fp32, kind="ExternalInput").ap()
    # identb: [128, 128] identity (bf16) as moving operand of bf16 transposes.
    identb = nc.dram_tensor("identb", [128, 128], bf16, kind="ExternalInput").ap()
    out = nc.dram_tensor("out", [bpc, C, n], fp32, kind="ExternalOutput").ap()

    with tile.TileContext(nc) as tc:
        with (
            tc.tile_pool(name="qpool", bufs=3) as qpool,
            tc.tile_pool(name="hipool", bufs=ntiles + 1) as hipool,
            tc.tile_pool(name="lopool", bufs=ntiles + 1) as lopool,
            tc.tile_pool(name="thpool", bufs=7) as thpool,
            tc.tile_pool(name="tlpool", bufs=3) as tlpool,
            tc.tile_pool(name="opool", bufs=2) as opool,
            tc.tile_pool(name="spool", bufs=1) as spool,
            tc.tile_pool(name="single", bufs=1) as single,
            tc.tile_pool(name="tps", bufs=3, space="PSUM") as tps_pool,
            tc.tile_pool(name="eps", bufs=1, space="PSUM") as eps_pool,
            tc.tile_pool(name="aps", bufs=1, space="PSUM") as aps_pool,
            tc.tile_pool(name="ops", bufs=2, space="PSUM") as ops_pool,
        ):
            ident_sb = single.tile([128, 64], fp32)
            nc.sync.dma_start(out=ident_sb, in_=ident)
            identb_sb = single.tile([128, 128], bf16)
            nc.sync.dma_start(out=identb_sb, in_=identb)
            gamma_sb = single.tile([128, 1], fp32)
            nc.sync.dma_start(out=gamma_sb, in_=gamma.to_broadcast((128, 1)))

            # Warmup transpose: absorbs the identb-DMA wait on PE so real
            # transposes carry a single wait (LDWEIGHTS allows one).
            warm = aps_pool.tile([128, 128], bf16, tag="atps")
            nc.tensor.transpose(warm, identb_sb, identb_sb)

            blocks_per_tile = tile_f // CHUNK  # n'-blocks per tile
            nblocks = ntiles * blocks_per_tile  # per batch (covers both halves)
            slabs_per_tile = tile_f // SLAB

            for b in range(bpc):
                # ---- Load + hi/lo split ----
                hitiles, lotiles = [], []
                for t in range(ntiles):
                    qt = qpool.tile([128, tile_f], fp32)
                    # Two DMAs per tile: [64, 2048] each has 64 outer DRAM
                    # rows, so HWDGE spreads descriptors across all 16 SDMA
                    # engines (a [2, 64, .] pattern lands on only 2).
                    nc.sync.dma_start(
                        out=qt[0:64, :], in_=x[b, :, t * tile_f : (t + 1) * tile_f]
                    )
                    nc.sync.dma_start(
                        out=qt[64:128, :],
                        in_=x[b, :, half + t * tile_f : half + (t + 1) * tile_f],
                    )
                    hi = hipool.tile([128, tile_f], bf16)
                    # Cast on the (otherwise idle) Scalar engine; the subtract
                    # stays on GpSimd so neither engine paces phase 1 alone.
                    nc.scalar.copy(out=hi, in_=qt)
                    lo = lopool.tile([128, tile_f], bf16)
                    nc.gpsimd.tensor_tensor(
                        out=lo, in0=qt, in1=hi, op=mybir.AluOpType.subtract
                    )
                    hitiles.append(hi)
                    lotiles.append(lo)

                # ---- Phase 1: transposes + pair-gram accumulation ----
                gacc = eps_pool.tile([128, 128], fp32, tag="gacc")
                xacc = None
                if USE_LO_ENERGY:
                    xacc = eps_pool.tile([128, 128], fp32, tag="xacc", name="xacc")
                # Software-pipelined: the lo chain (DMA -> ACT cast -> GpSimd
                # subtract, ~8.6us latency) lags one tile behind the hi chain
                # so PE always has ready hi-work while lo(t) is produced.
                tsb_his = {}  # t -> list of tsb tiles (kept for the lo pass)
                gcnt_g = 0
                gcnt_x = 0

                def emit_hi_pass(t):
                    nonlocal gcnt_g
                    hi = hitiles[t]
                    tsb_his[t] = []
                    groups = list(range(0, blocks_per_tile, TGROUP))
                    # All transposes first (copies overlap them), grams after:
                    # no PE round-trip stall on the PSUM->SBUF copy.
                    for c0 in groups:
                        ng = min(TGROUP, blocks_per_tile - c0)
                        tps_hi = tps_pool.tile(
                            [128, TGROUP * 128], bf16, tag="tps", name="tps_hi"
                        )
                        for i in range(ng):
                            cc = (c0 + i) * CHUNK
                            nc.tensor.transpose(
                                tps_hi[:, i * 128 : (i + 1) * 128],
                                hi[:, cc : cc + CHUNK],
                                identb_sb,
                            )
                        tsb_hi = thpool.tile(
                            [128, TGROUP * 128], bf16, tag="tsbh", name="tsb_hi"
                        )
                        hw = TGROUP * 64
                        nc.vector.tensor_copy(out=tsb_hi[:, :hw], in_=tps_hi[:, :hw])
                        nc.vector.tensor_copy(out=tsb_hi[:, hw:], in_=tps_hi[:, hw:])
                        tsb_his[t].append(tsb_hi)
                    for gi, c0 in enumerate(groups):
                        ng = min(TGROUP, blocks_per_tile - c0)
                        tsb_hi = tsb_his[t][gi]
                        for i in range(ng):
                            th = tsb_hi[:, i * 128 : (i + 1) * 128]
                            nc.tensor.matmul(
                                gacc,
                                lhsT=th,
                                rhs=th,
                                start=(gcnt_g == 0),
                                stop=(gcnt_g == nblocks - 1),
                            )
                            gcnt_g += 1

                def emit_lo_pass(t):
                    nonlocal gcnt_x
                    lo = lotiles[t]
                    groups = list(range(0, blocks_per_tile, TGROUP))
                    tsb_los = []
                    for c0 in groups:
                        ng = min(TGROUP, blocks_per_tile - c0)
                        tps_lo = tps_pool.tile(
                            [128, TGROUP * 128], bf16, tag="tps", name="tps_lo"
                        )
                        for i in range(ng):
                            cc = (c0 + i) * CHUNK
                            nc.tensor.transpose(
                                tps_lo[:, i * 128 : (i + 1) * 128],
                                lo[:, cc : cc + CHUNK],
                                identb_sb,
                            )
                        tsb_lo = tlpool.tile(
                            [128, TGROUP * 128], bf16, tag="tsbl", name="tsb_lo"
                        )
                        hw = TGROUP * 64
                        nc.vector.tensor_copy(out=tsb_lo[:, :hw], in_=tps_lo[:, :hw])
                        nc.vector.tensor_copy(out=tsb_lo[:, hw:], in_=tps_lo[:, hw:])
                        tsb_los.append(tsb_lo)
                    for gi, c0 in enumerate(groups):
                        ng = min(TGROUP, blocks_per_tile - c0)
                        tsb_hi = tsb_his[t][gi]
                        tsb_lo = tsb_los[gi]
                        for i in range(ng):
                            th = tsb_hi[:, i * 128 : (i + 1) * 128]
                            tl = tsb_lo[:, i * 128 : (i + 1) * 128]
                            nc.tensor.matmul(
                                xacc,
                                lhsT=th,
                                rhs=tl,
                                start=(gcnt_x == 0),
                                stop=(gcnt_x == nblocks - 1),
                            )
                            gcnt_x += 1
                    del tsb_his[t]

                LAG = 2  # lo chain lags the hi chain by this many tiles
                for t in range(ntiles + LAG):
                    if t < ntiles:
                        emit_hi_pass(t)
                    if USE_LO_ENERGY and t >= LAG:
                        emit_lo_pass(t - LAG)

                # ---- Combine energy, softmax, build phase-2 stationaries ----
                # Half-sums via matmul against stacked double identity:
                # E_hh = G[0:64,0:64] + G[64:128,64:128], Xs likewise.
                esb = spool.tile([128, 128], fp32)
                nc.vector.tensor_copy(out=esb[0:64, 0:64], in_=gacc[0:64, 0:64])
                nc.vector.tensor_copy(
                    out=esb[64:128, 0:64], in_=gacc[64:128, 64:128]
                )
                if USE_LO_ENERGY:
                    nc.vector.tensor_copy(
                        out=esb[0:64, 64:128], in_=xacc[0:64, 0:64]
                    )
                    nc.vector.tensor_copy(
                        out=esb[64:128, 64:128], in_=xacc[64:128, 64:128]
                    )
                msum = aps_pool.tile([64, 128], fp32, tag="atps")
                nc.tensor.matmul(
                    msum[:, 0:64],
                    lhsT=ident_sb,
                    rhs=esb[:, 0:64],
                    start=True,
                    stop=True,
                )
                if USE_LO_ENERGY:
                    nc.tensor.matmul(
                        msum[:, 64:128],
                        lhsT=ident_sb,
                        rhs=esb[:, 64:128],
                        start=True,
                        stop=True,
                    )
                msb = spool.tile([64, 128], fp32)
                nc.vector.tensor_copy(out=msb, in_=msum)
                efull = spool.tile([64, 64], fp32)
                if USE_LO_ENERGY:
                    # E = E_hh + Xs + Xs^T
                    xt = aps_pool.tile([64, 64], fp32, tag="atps")
                    nc.tensor.transpose(xt, msb[:, 64:128], ident_sb[0:64, :])
                    nc.vector.tensor_add(efull, msb[:, 0:64], msb[:, 64:128])
                    nc.vector.tensor_add(efull, efull, xt)
                else:
                    nc.vector.tensor_copy(out=efull, in_=msb[:, 0:64])

                # att = exp(rmin - E) / rowsum  (== softmax(rowmax(E)-E) rows)
                rmin = spool.tile([64, 1], fp32)
                nc.vector.tensor_reduce(
                    rmin, efull, axis=mybir.AxisListType.X, op=mybir.AluOpType.min
                )
                e2 = spool.tile([64, 128], fp32)
                nc.scalar.activation(
                    e2[:, 0:64],
                    efull,
                    mybir.ActivationFunctionType.Exp,
                    bias=rmin,
                    scale=-1.0,
                )
                ssum = spool.tile([64, 1], fp32)
                nc.vector.reduce_sum(ssum, e2[:, 0:64], axis=mybir.AxisListType.X)
                rsum = spool.tile([64, 1], fp32)
                nc.vector.reciprocal(rsum, ssum)
                att2 = spool.tile([64, 128], fp32)
                nc.vector.tensor_scalar_mul(att2[:, 0:64], e2[:, 0:64], rsum)
                nc.vector.tensor_copy(out=att2[:, 64:128], in_=att2[:, 0:64])
                # attT = [att^T ; att^T]
                atps = aps_pool.tile([128, 64], fp32, tag="atps")
                nc.tensor.transpose(atps, att2, ident_sb[0:64, :])
                # S = blockdiag(M^T, M^T), M = gamma*att + I; split bf16 hi/lo.
                ssb = spool.tile([128, 128], fp32)
                nc.vector.memset(ssb, 0.0)
                nc.vector.tensor_scalar_mul(
                    ssb[0:64, 0:64], atps[0:64, :], gamma_sb[0:64]
                )
                nc.vector.tensor_scalar_mul(
                    ssb[64:128, 64:128], atps[64:128, :], gamma_sb[64:128]
                )
                nc.vector.tensor_add(
                    ssb[0:64, 0:64], ssb[0:64, 0:64], ident_sb[0:64, :]
                )
                nc.vector.tensor_add(
                    ssb[64:128, 64:128], ssb[64:128, 64:128], ident_sb[64:128, :]
                )
                s_hi = spool.tile([128, 128], bf16)
                nc.vector.tensor_copy(out=s_hi, in_=ssb)
                s_lo = spool.tile([128, 128], bf16)
                nc.vector.tensor_tensor(
                    out=s_lo, in0=ssb, in1=s_hi, op=mybir.AluOpType.subtract
                )

                # ---- Phase 2: out = S_hi@hi + S_hi@lo + S_lo@hi ----
                for t in range(ntiles):
                    hi, lo = hitiles[t], lotiles[t]
                    osb = opool.tile([128, tile_f], fp32)
                    for s in range(slabs_per_tile):
                        sl = slice(s * SLAB, (s + 1) * SLAB)
                        ops = ops_pool.tile([128, SLAB], fp32)
                        nc.tensor.matmul(
                            ops, lhsT=s_hi, rhs=hi[:, sl], start=True, stop=False
                        )
                        nc.tensor.matmul(
                            ops,
                            lhsT=s_hi,
                            rhs=lo[:, sl],
                            start=False,
                            stop=not USE_LO_PHASE2,
                        )
                        if USE_LO_PHASE2:
                            nc.tensor.matmul(
                                ops,
                                lhsT=s_lo,
                                rhs=hi[:, sl],
                                start=False,
                                stop=True,
                            )
                        nc.vector.tensor_copy(out=osb[:, sl], in_=ops)
                    nc.scalar.dma_start(
                        out=out[b, :, t * tile_f : (t + 1) * tile_f],
                        in_=osb[0:64, :],
                    )
                    nc.scalar.dma_start(
                        out=out[b, :, half + t * tile_f : half + (t + 1) * tile_f],
                        in_=osb[64:128, :],
                    )

    if not nc.is_finalized():
        nc.finalize()
    return nc


def _make_ident():
    ident = np.zeros((128, 64), np.float32)
    ident[np.arange(64), np.arange(64)] = 1.0
    ident[64 + np.arange(64), np.arange(64)] = 1.0
    return ident


def _make_identb():
    import ml_dtypes

    return np.eye(128, dtype=ml_dtypes.bfloat16)


def _setup_trace_hook():
    """Register the axon NTFF profiling hook (the image's antenv lacks the
    axon_hooks shim module; rebuild it and wire it to libaxon_pjrt.so)."""
    import sys
    import types

    import antenv

    if "antenv.axon_hooks" not in sys.modules:
        mod = types.ModuleType("antenv.axon_hooks")
        mod._hook = None

        def set_axon_ntff_profile_hook(h):
            mod._hook = h

        def get_axon_ntff_profile_hook():
            return mod._hook

        mod.set_axon_ntff_profile_hook = set_axon_ntff_profile_hook
        mod.get_axon_ntff_profile_hook = get_axon_ntff_profile_hook
        sys.modules["antenv.axon_hooks"] = mod
        antenv.axon_hooks = mod

    hooks = sys.modules["antenv.axon_hooks"]
    if hooks.get_axon_ntff_profile_hook() is None:
        from trn_agent_boot.trn_boot import _ntff_profile_via_ctypes

        hooks.set_axon_ntff_profile_hook(
            _ntff_profile_via_ctypes("/opt/axon/libaxon_pjrt.so")
        )

    # No S3 in this container: keep profile artifacts local.
    import concourse.bass_utils as bu

    bu.upload_artifacts = lambda tmpdir: tmpdir


def run(x, gamma, trace=False, tmpdir=None):
    """Run the SPMD kernel on 8 cores. Returns (out, exec_time_ns_or_None)."""
    from concourse.bass_utils import run_bass_kernel_spmd

    if trace:
        try:
            _setup_trace_hook()
        except Exception as e:  # tracing is best-effort; execution still works
            print("trace setup failed:", e)

    x = np.ascontiguousarray(np.asarray(x, dtype=np.float32))
    gamma = np.ascontiguousarray(np.asarray(gamma, dtype=np.float32))
    assert x.shape == (B, C, H, W), x.shape

    nc = build_cam_program()
    ident = _make_ident()
    identb = _make_identb()
    xr = x.reshape(NCORES, BPC, C, N)
    in_maps = [
        {
            "x": np.ascontiguousarray(xr[i]),
            "gamma": gamma,
            "ident": ident,
            "identb": identb,
        }
        for i in range(NCORES)
    ]
    res = run_bass_kernel_spmd(
        nc, in_maps, core_ids=list(range(NCORES)), trace=trace, tmpdir=tmpdir
    )
    outs = np.stack([np.asarray(res.results[i]["out"]) for i in range(NCORES)])
    y = outs.reshape(B, C, H, W).astype(np.float32)
    return y, res.exec_time_ns


def kernel(x, gamma):
    y, _ = run(x, gamma)
    return y


# revision 31
# speedup vs baseline: 1.1149x; 1.1149x over previous
"""CAM (channel attention) module kernel for Trainium2, SPMD over 8 NeuronCores.

Reference computation (per batch b):
    q = x[b].reshape(C, N)                  # C=64, N=H*W=65536
    energy = q @ q.T                        # [C, C]
    att = softmax(rowmax(energy) - energy)  # == softmax(-energy) rows
    out[b] = gamma * (att @ q) + x[b]

Sharding: data-parallel over batch, 2 batches per core, no cross-core comm.

Per-core design (v2, bf16 hi/lo split for PE speed, fp32-grade accuracy):

  Layout: q2 [128, 32768] fp32 where partition p = h*64 + c (h = n-half,
  c = channel), streamed in [128, 2048] tiles (two [64, 2048] DMAs each so
  the HWDGE spreads descriptors over all 16 SDMA engines).

  Split: hi = bf16(q), lo = bf16(q - hi) on GpSimd (idle engine), after
  which the fp32 tile is released (hi+lo reconstructs q to ~2^-18).

  Phase 1 (energy): PE-transpose [128, 128] bf16 blocks of hi/lo (one op
  covers both n-halves), stage groups of 8 in a PSUM bank, DVE-copy to
  SBUF, then bf16 pair-gram matmuls accumulate
      G  += Thi^T @ Thi   (diag blocks = per-half energies)
      X  += Thi^T @ Tlo   (hi-lo cross term; lo.lo term is negligible)
  E = (G00+G11) + (X00+X11) + (X00+X11)^T, where the half sums are done
  with one matmul against the stacked double identity.

  Softmax: att = exp(rmin(E) - E) / rowsum (shift-invariant == reference).
  Build S = blockdiag(M^T, M^T), M = gamma*att + I, split S into bf16
  hi/lo. The identity on the diagonal carries the residual.

  Phase 2: out_slab = S_hi@hi + S_hi@lo + S_lo@hi (3 bf16 matmuls into one
  PSUM bank), DVE copy to staging, two [64, 2048] DMAs out per tile.
"""

import os

import numpy as np

import concourse.bass as bass
import concourse.tile as tile
from concourse import bacc, mybir

# Problem constants (hardcoded per harness contract).
B, C, H, W = 16, 64, 256, 256
N = H * W  # 65536
NCORES = 8
BPC = B // NCORES  # batches per core

# Tunables.
TILE_F = 2048  # free width of a q2 tile
CHUNK = 128  # n'-block width (covers both halves per transpose/gram)
TGROUP = 8  # transposed blocks per PSUM bank group
SLAB = 512  # phase-2 moving width
USE_LO_ENERGY = os.environ.get("CAM_LO_ENERGY", "1") == "1"
USE_LO_PHASE2 = os.environ.get("CAM_LO_PHASE2", "0") == "1"


def build_cam_program(n=N, bpc=BPC, tile_f=TILE_F):
    """Build the single-core Bass program (same program runs on all cores)."""
    half = n // 2
    ntiles = half // tile_f
    fp32 = mybir.dt.float32
    bf16 = mybir.dt.bfloat16

    # Bacc (not plain Bass): its finalize() runs move_matmul_waits_to_ldweights
    # and generate_event_semaphores, which split multi-sem waits down to the
    # TRN2 limit of one embedded wait per instruction.
    nc = bacc.Bacc("TRN2", target_bir_lowering=False, debug=False)
    x = nc.dram_tensor("x", [bpc, C, n], fp32, kind="ExternalInput").ap()
    gamma = nc.dram_tensor("gamma", [1], fp32, kind="ExternalInput").ap()
    # ident2: [128, 64] stacked double identity (fp32) for half-sum matmuls.
    ident = nc.dram_tensor("ident", [128, 64], fp32, kind="ExternalInput").ap()
    # identb: [128, 128] identity (bf16) as moving operand of bf16 transposes.
    identb = nc.dram_tensor("identb", [128, 128], bf16, kind="ExternalInput").ap()
    out = nc.dram_tensor("out", [bpc, C, n], fp32, kind="ExternalOutput").ap()

    with tile.TileContext(nc) as tc:
        with (
            tc.tile_pool(name="qpool", bufs=3) as qpool,
            tc.tile_pool(name="hipool", bufs=ntiles + 1) as hipool,
            tc.tile_pool(name="lopool", bufs=ntiles + 1) as lopool,
            tc.tile_pool(name="thpool", bufs=7) as thpool,
            tc.tile_pool(name="tlpool", bufs=3) as tlpool,
            tc.tile_pool(name="opool", bufs=2) as opool,
            tc.tile_pool(name="spool", bufs=1) as spool,
            tc.tile_pool(name="single", bufs=1) as single,
            tc.tile_pool(name="tps", bufs=3, space="PSUM") as tps_pool,
            tc.tile_pool(name="eps", bufs=1, space="PSUM") as eps_pool,
            tc.tile_pool(name="aps", bufs=1, space="PSUM") as aps_pool,
            tc.tile_pool(name="ops", bufs=2, space="PSUM") as ops_pool,
        ):
            ident_sb = single.tile([128, 64], fp32)
            nc.sync.dma_start(out=ident_sb, in_=ident)
            identb_sb = single.tile([128, 128], bf16)
            nc.sync.dma_start(out=identb_sb, in_=identb)
            gamma_sb = single.tile([128, 1], fp32)
            nc.sync.dma_start(out=gamma_sb, in_=gamma.to_broadcast((128, 1)))

            # Warmup transpose: absorbs the identb-DMA wait on PE so real
            # transposes carry a single wait (LDWEIGHTS allows one).
            warm = aps_pool.tile([128, 128], bf16, tag="atps")
            nc.tensor.transpose(warm, identb_sb, identb_sb)

            blocks_per_tile = tile_f // CHUNK  # n'-blocks per tile
            nblocks = ntiles * blocks_per_tile  # per batch (covers both halves)
            slabs_per_tile = tile_f // SLAB

            for b in range(bpc):
                # ---- Load + hi/lo split ----
                hitiles, lotiles = [], []
                for t in range(ntiles):
                    qt = qpool.tile([128, tile_f], fp32)
                    # Two DMAs per tile: [64, 2048] each has 64 outer DRAM
                    # rows, so HWDGE spreads descriptors across all 16 SDMA
                    # engines (a [2, 64, .] pattern lands on only 2).
                    nc.sync.dma_start(
                        out=qt[0:64, :], in_=x[b, :, t * tile_f : (t + 1) * tile_f]
                    )
                    nc.sync.dma_start(
                        out=qt[64:128, :],
                        in_=x[b, :, half + t * tile_f : half + (t + 1) * tile_f],
                    )
                    hi = hipool.tile([128, tile_f], bf16)
                    # Cast on the (otherwise idle) Scalar engine; the subtract
                    # stays on GpSimd so neither engine paces phase 1 alone.
                    nc.scalar.copy(out=hi, in_=qt)
                    lo = lopool.tile([128, tile_f], bf16)
                    nc.gpsimd.tensor_tensor(
                        out=lo, in0=qt, in1=hi, op=mybir.AluOpType.subtract
                    )
                    hitiles.append(hi)
                    lotiles.append(lo)

                # ---- Phase 1: transposes + pair-gram accumulation ----
                gacc = eps_pool.tile([128, 128], fp32, tag="gacc")
                xacc = None
                if USE_LO_ENERGY:
                    xacc = eps_pool.tile([128, 128], fp32, tag="xacc", name="xacc")
                # Software-pipelined: the lo chain (DMA -> ACT cast -> GpSimd
                # subtract, ~8.6us latency) lags one tile behind the hi chain
                # so PE always has ready hi-work while lo(t) is produced.
                tsb_his = {}  # t -> list of tsb tiles (kept for the lo pass)
                gcnt_g = 0
                gcnt_x = 0

                def emit_hi_pass(t):
                    nonlocal gcnt_g
                    hi = hitiles[t]
                    tsb_his[t] = []
                    groups = list(range(0, blocks_per_tile, TGROUP))
                    # All transposes first (copies overlap them), grams after:
                    # no PE round-trip stall on the PSUM->SBUF copy.
                    for c0 in groups:
                        ng = min(TGROUP, blocks_per_tile - c0)
                        tps_hi = tps_pool.tile(
                            [128, TGROUP * 128], bf16, tag="tps", name="tps_hi"
                        )
                        for i in range(ng):
                            cc = (c0 + i) * CHUNK
                            nc.tensor.transpose(
                                tps_hi[:, i * 128 : (i + 1) * 128],
                                hi[:, cc : cc + CHUNK],
                                identb_sb,
                            )
                        tsb_hi = thpool.tile(
                            [128, TGROUP * 128], bf16, tag="tsbh", name="tsb_hi"
                        )
                        hw = TGROUP * 64
                        nc.vector.tensor_copy(out=tsb_hi[:, :hw], in_=tps_hi[:, :hw])
                        nc.vector.tensor_copy(out=tsb_hi[:, hw:], in_=tps_hi[:, hw:])
                        tsb_his[t].append(tsb_hi)
                    for gi, c0 in enumerate(groups):
                        ng = min(TGROUP, blocks_per_tile - c0)
                        tsb_hi = tsb_his[t][gi]
                        for i in range(ng):
                            th = tsb_hi[:, i * 128 : (i + 1) * 128]
                            nc.tensor.matmul(
                                gacc,
                                lhsT=th,
                                rhs=th,
                                start=(gcnt_g == 0),
                                stop=(gcnt_g == nblocks - 1),
                            )
                            gcnt_g += 1

                def emit_lo_pass(t):
                    nonlocal gcnt_x
                    lo = lotiles[t]
                    groups = list(range(0, blocks_per_tile, TGROUP))
                    tsb_los = []
                    for c0 in groups:
                        ng = min(TGROUP, blocks_per_tile - c0)
                        tps_lo = tps_pool.tile(
                            [128, TGROUP * 128], bf16, tag="tps", name="tps_lo"
                        )
                        for i in range(ng):
                            cc = (c0 + i) * CHUNK
                            nc.tensor.transpose(
                                tps_lo[:, i * 128 : (i + 1) * 128],
                                lo[:, cc : cc + CHUNK],
                                identb_sb,
                            )
                        tsb_lo = tlpool.tile(
                            [128, TGROUP * 128], bf16, tag="tsbl", name="tsb_lo"
                        )
                        hw = TGROUP * 64
                        nc.vector.tensor_copy(out=tsb_lo[:, :hw], in_=tps_lo[:, :hw])
                        nc.vector.tensor_copy(out=tsb_lo[:, hw:], in_=tps_lo[:, hw:])
                        tsb_los.append(tsb_lo)
                    for gi, c0 in enumerate(groups):
                        ng = min(TGROUP, blocks_per_tile - c0)
                        tsb_hi = tsb_his[t][gi]
                        tsb_lo = tsb_los[gi]
                        for i in range(ng):
                            th = tsb_hi[:, i * 128 : (i + 1) * 128]
                            tl = tsb_lo[:, i * 128 : (i + 1) * 128]
                            nc.tensor.matmul(
                                xacc,
                                lhsT=th,
                                rhs=tl,
                                start=(gcnt_x == 0),
                                stop=(gcnt_x == nblocks - 1),
                            )
                            gcnt_x += 1
                    del tsb_his[t]

                LAG = 2  # lo chain lags the hi chain by this many tiles
                for t in range(ntiles + LAG):
                    if t < ntiles:
                        emit_hi_pass(t)
                    if USE_LO_ENERGY and t >= LAG:
                        emit_lo_pass(t - LAG)

                # ---- Combine energy, softmax, build phase-2 stationaries ----
                # Half-sums via matmul against stacked double identity:
                # E_hh = G[0:64,0:64] + G[64:128,64:128], Xs likewise.
                esb = spool.tile([128, 128], fp32)
                nc.vector.tensor_copy(out=esb[0:64, 0:64], in_=gacc[0:64, 0:64])
                nc.vector.tensor_copy(
                    out=esb[64:128, 0:64], in_=gacc[64:128, 64:128]
                )
                if USE_LO_ENERGY:
                    nc.vector.tensor_copy(
                        out=esb[0:64, 64:128], in_=xacc[0:64, 0:64]
                    )
                    nc.vector.tensor_copy(
                        out=esb[64:128, 64:128], in_=xacc[64:128, 64:128]
                    )
                msum = aps_pool.tile([64, 128], fp32, tag="atps")
                nc.tensor.matmul(
                    msum[:, 0:64],
                    lhsT=ident_sb,
                    rhs=esb[:, 0:64],
                    start=True,
                    stop=True,
                )
                if USE_LO_ENERGY:
                    nc.tensor.matmul(
                        msum[:, 64:128],
                        lhsT=ident_sb,
                        rhs=esb[:, 64:128],
                        start=True,
                        stop=True,
                    )
                msb = spool.tile([64, 128], fp32)
                nc.vector.tensor_copy(out=msb, in_=msum)
                efull = spool.tile([64, 64], fp32)
                if USE_LO_ENERGY:
                    # E = E_hh + Xs + Xs^T
                    xt = aps_pool.tile([64, 64], fp32, tag="atps")
                    nc.tensor.transpose(xt, msb[:, 64:128], ident_sb[0:64, :])
                    nc.vector.tensor_add(efull, msb[:, 0:64], msb[:, 64:128])
                    nc.vector.tensor_add(efull, efull, xt)
                else:
                    nc.vector.tensor_copy(out=efull, in_=msb[:, 0:64])

                # att = exp(rmin - E) / rowsum  (== softmax(rowmax(E)-E) rows)
                rmin = spool.tile([64, 1], fp32)
                nc.vector.tensor_reduce(
                    rmin, efull, axis=mybir.AxisListType.X, op=mybir.AluOpType.min
                )
                e2 = spool.tile([64, 128], fp32)
                nc.scalar.activation(
                    e2[:, 0:64],
                    efull,
                    mybir.ActivationFunctionType.Exp,
                    bias=rmin,
                    scale=-1.0,
                )
                ssum = spool.tile([64, 1], fp32)
                nc.vector.reduce_sum(ssum, e2[:, 0:64], axis=mybir.AxisListType.X)
                rsum = spool.tile([64, 1], fp32)
                nc.vector.reciprocal(rsum, ssum)
                att2 = spool.tile([64, 128], fp32)
                nc.vector.tensor_scalar_mul(att2[:, 0:64], e2[:, 0:64], rsum)
                nc.vector.tensor_copy(out=att2[:, 64:128], in_=att2[:, 0:64])
                # attT = [att^T ; att^T]
                atps = aps_pool.tile([128, 64], fp32, tag="atps")
                nc.tensor.transpose(atps, att2, ident_sb[0:64, :])
                # S = blockdiag(M^T, M^T), M = gamma*att + I; split bf16 hi/lo.
                ssb = spool.tile([128, 128], fp32)
                nc.vector.memset(ssb, 0.0)
                nc.vector.tensor_scalar_mul(
                    ssb[0:64, 0:64], atps[0:64, :], gamma_sb[0:64]
                )
                nc.vector.tensor_scalar_mul(
                    ssb[64:128, 64:128], atps[64:128, :], gamma_sb[64:128]
                )
                nc.vector.tensor_add(
                    ssb[0:64, 0:64], ssb[0:64, 0:64], ident_sb[0:64, :]
                )
                nc.vector.tensor_add(
                    ssb[64:128, 64:128], ssb[64:128, 64:128], ident_sb[64:128, :]
                )
                s_hi = spool.tile([128, 128], bf16)
                nc.vector.tensor_copy(out=s_hi, in_=ssb)
                s_lo = spool.tile([128, 128], bf16)
                nc.vector.tensor_tensor(
                    out=s_lo, in0=ssb, in1=s_hi, op=mybir.AluOpType.subtract
                )

                # ---- Phase 2: out = S_hi@hi + S_hi@lo + S_lo@hi ----
                for t in range(ntiles):
                    hi, lo = hitiles[t], lotiles[t]
                    osb = opool.tile([128, tile_f], fp32)
                    for s in range(slabs_per_tile):
                        sl = slice(s * SLAB, (s + 1) * SLAB)
                        ops = ops_pool.tile([128, SLAB], fp32)
                        nc.tensor.matmul(
                            ops, lhsT=s_hi, rhs=hi[:, sl], start=True, stop=False
                        )
                        nc.tensor.matmul(
                            ops,
                            lhsT=s_hi,
                            rhs=lo[:, sl],
                            start=False,
                            stop=not USE_LO_PHASE2,
                        )
                        if USE_LO_PHASE2:
                            nc.tensor.matmul(
                                ops,
                                lhsT=s_lo,
                                rhs=hi[:, sl],
                                start=False,
                                stop=True,
                            )
                        # Alternate DVE/ACT so phase-2 copies don't queue
                        # behind the next batch's phase-1 copies on DVE.
                        if s % 2 == 0:
                            nc.vector.tensor_copy(out=osb[:, sl], in_=ops)
                        else:
                            nc.scalar.copy(out=osb[:, sl], in_=ops)
                    nc.scalar.dma_start(
                        out=out[b, :, t * tile_f : (t + 1) * tile_f],
                        in_=osb[0:64, :],
                    )
                    nc.scalar.dma_start(
                        out=out[b, :, half + t * tile_f : half + (t + 1) * tile_f],
                        in_=osb[64:128, :],
                    )

    if not nc.is_finalized():
        nc.finalize()
    return nc


def _make_ident():
    ident = np.zeros((128, 64), np.float32)
    ident[np.arange(64), np.arange(64)] = 1.0
    ident[64 + np.arange(64), np.arange(64)] = 1.0
    return ident


def _make_identb():
    import ml_dtypes

    return np.eye(128, dtype=ml_dtypes.bfloat16)


def _setup_trace_hook():
    """Register the axon NTFF profiling hook (the image's antenv lacks the
    axon_hooks shim module; rebuild it and wire it to libaxon_pjrt.so)."""
    import sys
    import types

    import antenv

    if "antenv.axon_hooks" not in sys.modules:
        mod = types.ModuleType("antenv.axon_hooks")
        mod._hook = None

        def set_axon_ntff_profile_hook(h):
            mod._hook = h

        def get_axon_ntff_profile_hook():
            return mod._hook

        mod.set_axon_ntff_profile_hook = set_axon_ntff_profile_hook
        mod.get_axon_ntff_profile_hook = get_axon_ntff_profile_hook
        sys.modules["antenv.axon_hooks"] = mod
        antenv.axon_hooks = mod

    hooks = sys.modules["antenv.axon_hooks"]
    if hooks.get_axon_ntff_profile_hook() is None:
        from trn_agent_boot.trn_boot import _ntff_profile_via_ctypes

        hooks.set_axon_ntff_profile_hook(
            _ntff_profile_via_ctypes("/opt/axon/libaxon_pjrt.so")
        )

    # No S3 in this container: keep profile artifacts local.
    import concourse.bass_utils as bu

    bu.upload_artifacts = lambda tmpdir: tmpdir


def run(x, gamma, trace=False, tmpdir=None):
    """Run the SPMD kernel on 8 cores. Returns (out, exec_time_ns_or_None)."""
    from concourse.bass_utils import run_bass_kernel_spmd

    if trace:
        try:
            _setup_trace_hook()
        except Exception as e:  # tracing is best-effort; execution still works
            print("trace setup failed:", e)

    x = np.ascontiguousarray(np.asarray(x, dtype=np.float32))
    gamma = np.ascontiguousarray(np.asarray(gamma, dtype=np.float32))
    assert x.shape == (B, C, H, W), x.shape

    nc = build_cam_program()
    ident = _make_ident()
    identb = _make_identb()
    xr = x.reshape(NCORES, BPC, C, N)
    in_maps = [
        {
            "x": np.ascontiguousarray(xr[i]),
            "gamma": gamma,
            "ident": ident,
            "identb": identb,
        }
        for i in range(NCORES)
    ]
    res = run_bass_kernel_spmd(
        nc, in_maps, core_ids=list(range(NCORES)), trace=trace, tmpdir=tmpdir
    )
    outs = np.stack([np.asarray(res.results[i]["out"]) for i in range(NCORES)])
    y = outs.reshape(B, C, H, W).astype(np.float32)
    return y, res.exec_time_ns


def kernel(x, gamma):
    y, _ = run(x, gamma)
    return y


# revision 35
# speedup vs baseline: 1.1335x; 1.0166x over previous
"""CAM (channel attention) module kernel for Trainium2, SPMD over 8 NeuronCores.

Reference computation (per batch b):
    q = x[b].reshape(C, N)                  # C=64, N=H*W=65536
    energy = q @ q.T                        # [C, C]
    att = softmax(rowmax(energy) - energy)  # == softmax(-energy) rows
    out[b] = gamma * (att @ q) + x[b]

Sharding: data-parallel over batch, 2 batches per core, no cross-core comm.

Per-core design (v2, bf16 hi/lo split for PE speed, fp32-grade accuracy):

  Layout: q2 [128, 32768] fp32 where partition p = h*64 + c (h = n-half,
  c = channel), streamed in [128, 2048] tiles (two [64, 2048] DMAs each so
  the HWDGE spreads descriptors over all 16 SDMA engines).

  Split: hi = bf16(q), lo = bf16(q - hi) on GpSimd (idle engine), after
  which the fp32 tile is released (hi+lo reconstructs q to ~2^-18).

  Phase 1 (energy): PE-transpose [128, 128] bf16 blocks of hi/lo (one op
  covers both n-halves), stage groups of 8 in a PSUM bank, DVE-copy to
  SBUF, then bf16 pair-gram matmuls accumulate
      G  += Thi^T @ Thi   (diag blocks = per-half energies)
      X  += Thi^T @ Tlo   (hi-lo cross term; lo.lo term is negligible)
  E = (G00+G11) + (X00+X11) + (X00+X11)^T, where the half sums are done
  with one matmul against the stacked double identity.

  Softmax: att = exp(rmin(E) - E) / rowsum (shift-invariant == reference).
  Build S = blockdiag(M^T, M^T), M = gamma*att + I, split S into bf16
  hi/lo. The identity on the diagonal carries the residual.

  Phase 2: out_slab = S_hi@hi + S_hi@lo + S_lo@hi (3 bf16 matmuls into one
  PSUM bank), DVE copy to staging, two [64, 2048] DMAs out per tile.
"""

import os

import numpy as np

import concourse.bass as bass
import concourse.tile as tile
from concourse import bacc, mybir

# Problem constants (hardcoded per harness contract).
B, C, H, W = 16, 64, 256, 256
N = H * W  # 65536
NCORES = 8
BPC = B // NCORES  # batches per core

# Tunables.
TILE_F = 2048  # free width of a q2 tile
CHUNK = 128  # n'-block width (covers both halves per transpose/gram)
TGROUP = 8  # transposed blocks per PSUM bank group
SLAB = 512  # phase-2 moving width
USE_LO_ENERGY = os.environ.get("CAM_LO_ENERGY", "1") == "1"
USE_LO_PHASE2 = os.environ.get("CAM_LO_PHASE2", "0") == "1"


def build_cam_program(n=N, bpc=BPC, tile_f=TILE_F):
    """Build the single-core Bass program (same program runs on all cores)."""
    half = n // 2
    ntiles = half // tile_f
    fp32 = mybir.dt.float32
    bf16 = mybir.dt.bfloat16

    # Bacc (not plain Bass): its finalize() runs move_matmul_waits_to_ldweights
    # and generate_event_semaphores, which split multi-sem waits down to the
    # TRN2 limit of one embedded wait per instruction.
    nc = bacc.Bacc("TRN2", target_bir_lowering=False, debug=False)
    x = nc.dram_tensor("x", [bpc, C, n], fp32, kind="ExternalInput").ap()
    gamma = nc.dram_tensor("gamma", [1], fp32, kind="ExternalInput").ap()
    # ident2: [128, 64] stacked double identity (fp32) for half-sum matmuls.
    ident = nc.dram_tensor("ident", [128, 64], fp32, kind="ExternalInput").ap()
    # identb: [128, 128] identity (bf16) as moving operand of bf16 transposes.
    identb = nc.dram_tensor("identb", [128, 128], bf16, kind="ExternalInput").ap()
    out = nc.dram_tensor("out", [bpc, C, n], fp32, kind="ExternalOutput").ap()

    with tile.TileContext(nc) as tc:
        with (
            tc.tile_pool(name="qpool", bufs=3) as qpool,
            tc.tile_pool(name="hipool", bufs=ntiles + 1) as hipool,
            tc.tile_pool(name="lopool", bufs=ntiles + 1) as lopool,
            tc.tile_pool(name="thpool", bufs=7) as thpool,
            tc.tile_pool(name="tlpool", bufs=3) as tlpool,
            tc.tile_pool(name="opool", bufs=2) as opool,
            tc.tile_pool(name="spool", bufs=1) as spool,
            tc.tile_pool(name="single", bufs=1) as single,
            tc.tile_pool(name="tps", bufs=4, space="PSUM") as tps_pool,
            tc.tile_pool(name="eps", bufs=1, space="PSUM") as eps_pool,
            tc.tile_pool(name="ops", bufs=2, space="PSUM") as ops_pool,
        ):
            ident_sb = single.tile([128, 64], fp32)
            nc.sync.dma_start(out=ident_sb, in_=ident)
            identb_sb = single.tile([128, 128], bf16)
            nc.sync.dma_start(out=identb_sb, in_=identb)
            gamma_sb = single.tile([128, 1], fp32)
            nc.sync.dma_start(out=gamma_sb, in_=gamma.to_broadcast((128, 1)))

            # Warmup transpose: absorbs the identb-DMA wait on PE so real
            # transposes carry a single wait (LDWEIGHTS allows one).
            warm = ops_pool.tile([128, 128], bf16, tag="ops", name="warm")
            nc.tensor.transpose(warm, identb_sb, identb_sb)

            blocks_per_tile = tile_f // CHUNK  # n'-blocks per tile
            nblocks = ntiles * blocks_per_tile  # per batch (covers both halves)
            slabs_per_tile = tile_f // SLAB

            for b in range(bpc):
                # ---- Load + hi/lo split ----
                hitiles, lotiles = [], []
                for t in range(ntiles):
                    qt = qpool.tile([128, tile_f], fp32)
                    # Two DMAs per tile: [64, 2048] each has 64 outer DRAM
                    # rows, so HWDGE spreads descriptors across all 16 SDMA
                    # engines (a [2, 64, .] pattern lands on only 2).
                    nc.sync.dma_start(
                        out=qt[0:64, :], in_=x[b, :, t * tile_f : (t + 1) * tile_f]
                    )
                    nc.sync.dma_start(
                        out=qt[64:128, :],
                        in_=x[b, :, half + t * tile_f : half + (t + 1) * tile_f],
                    )
                    hi = hipool.tile([128, tile_f], bf16)
                    # Cast on the (otherwise idle) Scalar engine; the subtract
                    # stays on GpSimd so neither engine paces phase 1 alone.
                    nc.scalar.copy(out=hi, in_=qt)
                    lo = lopool.tile([128, tile_f], bf16)
                    nc.gpsimd.tensor_tensor(
                        out=lo, in0=qt, in1=hi, op=mybir.AluOpType.subtract
                    )
                    hitiles.append(hi)
                    lotiles.append(lo)

                # ---- Phase 1: transposes + pair-gram accumulation ----
                gacc = eps_pool.tile([128, 128], fp32, tag="gacc")
                xacc = None
                if USE_LO_ENERGY:
                    xacc = eps_pool.tile([128, 128], fp32, tag="xacc", name="xacc")
                # Software-pipelined: the lo chain (DMA -> ACT cast -> GpSimd
                # subtract, ~8.6us latency) lags one tile behind the hi chain
                # so PE always has ready hi-work while lo(t) is produced.
                tsb_his = {}  # t -> list of tsb tiles (kept for the lo pass)
                gcnt_g = 0
                gcnt_x = 0

                def emit_hi_pass(t):
                    nonlocal gcnt_g
                    hi = hitiles[t]
                    tsb_his[t] = []
                    groups = list(range(0, blocks_per_tile, TGROUP))
                    # All transposes first (copies overlap them), grams after:
                    # no PE round-trip stall on the PSUM->SBUF copy.
                    for c0 in groups:
                        ng = min(TGROUP, blocks_per_tile - c0)
                        tps_hi = tps_pool.tile(
                            [128, TGROUP * 128], bf16, tag="tps", name="tps_hi"
                        )
                        for i in range(ng):
                            cc = (c0 + i) * CHUNK
                            nc.tensor.transpose(
                                tps_hi[:, i * 128 : (i + 1) * 128],
                                hi[:, cc : cc + CHUNK],
                                identb_sb,
                            )
                        tsb_hi = thpool.tile(
                            [128, TGROUP * 128], bf16, tag="tsbh", name="tsb_hi"
                        )
                        hw = TGROUP * 64
                        nc.vector.tensor_copy(out=tsb_hi[:, :hw], in_=tps_hi[:, :hw])
                        nc.vector.tensor_copy(out=tsb_hi[:, hw:], in_=tps_hi[:, hw:])
                        tsb_his[t].append(tsb_hi)
                    for gi, c0 in enumerate(groups):
                        ng = min(TGROUP, blocks_per_tile - c0)
                        tsb_hi = tsb_his[t][gi]
                        for i in range(ng):
                            th = tsb_hi[:, i * 128 : (i + 1) * 128]
                            nc.tensor.matmul(
                                gacc,
                                lhsT=th,
                                rhs=th,
                                start=(gcnt_g == 0),
                                stop=(gcnt_g == nblocks - 1),
                            )
                            gcnt_g += 1

                def emit_lo_pass(t):
                    nonlocal gcnt_x
                    lo = lotiles[t]
                    groups = list(range(0, blocks_per_tile, TGROUP))
                    tsb_los = []
                    for c0 in groups:
                        ng = min(TGROUP, blocks_per_tile - c0)
                        tps_lo = tps_pool.tile(
                            [128, TGROUP * 128], bf16, tag="tps", name="tps_lo"
                        )
                        for i in range(ng):
                            cc = (c0 + i) * CHUNK
                            nc.tensor.transpose(
                                tps_lo[:, i * 128 : (i + 1) * 128],
                                lo[:, cc : cc + CHUNK],
                                identb_sb,
                            )
                        tsb_lo = tlpool.tile(
                            [128, TGROUP * 128], bf16, tag="tsbl", name="tsb_lo"
                        )
                        hw = TGROUP * 64
                        # lo copies ride ACT so they don't queue behind the hi
                        # copies (and the other batch's phase-2 copies) on DVE.
                        nc.scalar.copy(out=tsb_lo[:, :hw], in_=tps_lo[:, :hw])
                        nc.scalar.copy(out=tsb_lo[:, hw:], in_=tps_lo[:, hw:])
                        tsb_los.append(tsb_lo)
                    for gi, c0 in enumerate(groups):
                        ng = min(TGROUP, blocks_per_tile - c0)
                        tsb_hi = tsb_his[t][gi]
                        tsb_lo = tsb_los[gi]
                        for i in range(ng):
                            th = tsb_hi[:, i * 128 : (i + 1) * 128]
                            tl = tsb_lo[:, i * 128 : (i + 1) * 128]
                            nc.tensor.matmul(
                                xacc,
                                lhsT=th,
                                rhs=tl,
                                start=(gcnt_x == 0),
                                stop=(gcnt_x == nblocks - 1),
                            )
                            gcnt_x += 1
                    del tsb_his[t]

                LAG = 2  # lo chain lags the hi chain by this many tiles
                for t in range(ntiles + LAG):
                    if t < ntiles:
                        emit_hi_pass(t)
                    if USE_LO_ENERGY and t >= LAG:
                        emit_lo_pass(t - LAG)

                # ---- Combine energy, softmax, build phase-2 stationaries ----
                # Half-sums via matmul against stacked double identity:
                # E_hh = G[0:64,0:64] + G[64:128,64:128], Xs likewise.
                esb = spool.tile([128, 128], fp32)
                nc.vector.tensor_copy(out=esb[0:64, 0:64], in_=gacc[0:64, 0:64])
                nc.vector.tensor_copy(
                    out=esb[64:128, 0:64], in_=gacc[64:128, 64:128]
                )
                if USE_LO_ENERGY:
                    nc.vector.tensor_copy(
                        out=esb[0:64, 64:128], in_=xacc[0:64, 0:64]
                    )
                    nc.vector.tensor_copy(
                        out=esb[64:128, 64:128], in_=xacc[64:128, 64:128]
                    )
                msum = ops_pool.tile([64, 128], fp32, tag="ops", name="msum")
                nc.tensor.matmul(
                    msum[:, 0:64],
                    lhsT=ident_sb,
                    rhs=esb[:, 0:64],
                    start=True,
                    stop=True,
                )
                if USE_LO_ENERGY:
                    nc.tensor.matmul(
                        msum[:, 64:128],
                        lhsT=ident_sb,
                        rhs=esb[:, 64:128],
                        start=True,
                        stop=True,
                    )
                msb = spool.tile([64, 128], fp32)
                nc.vector.tensor_copy(out=msb, in_=msum)
                efull = spool.tile([64, 64], fp32)
                if USE_LO_ENERGY:
                    # E = E_hh + Xs + Xs^T
                    xt = ops_pool.tile([64, 64], fp32, tag="ops", name="xt")
                    nc.tensor.transpose(xt, msb[:, 64:128], ident_sb[0:64, :])
                    nc.vector.tensor_add(efull, msb[:, 0:64], msb[:, 64:128])
                    nc.vector.tensor_add(efull, efull, xt)
                else:
                    nc.vector.tensor_copy(out=efull, in_=msb[:, 0:64])

                # att = exp(rmin - E) / rowsum  (== softmax(rowmax(E)-E) rows)
                rmin = spool.tile([64, 1], fp32)
                nc.vector.tensor_reduce(
                    rmin, efull, axis=mybir.AxisListType.X, op=mybir.AluOpType.min
                )
                e2 = spool.tile([64, 128], fp32)
                nc.scalar.activation(
                    e2[:, 0:64],
                    efull,
                    mybir.ActivationFunctionType.Exp,
                    bias=rmin,
                    scale=-1.0,
                )
                ssum = spool.tile([64, 1], fp32)
                nc.vector.reduce_sum(ssum, e2[:, 0:64], axis=mybir.AxisListType.X)
                rsum = spool.tile([64, 1], fp32)
                nc.vector.reciprocal(rsum, ssum)
                att2 = spool.tile([64, 128], fp32)
                nc.vector.tensor_scalar_mul(att2[:, 0:64], e2[:, 0:64], rsum)
                nc.vector.tensor_copy(out=att2[:, 64:128], in_=att2[:, 0:64])
                # attT = [att^T ; att^T]
                atps = ops_pool.tile([128, 64], fp32, tag="ops", name="atps")
                nc.tensor.transpose(atps, att2, ident_sb[0:64, :])
                # S = blockdiag(M^T, M^T), M = gamma*att + I; split bf16 hi/lo.
                ssb = spool.tile([128, 128], fp32)
                nc.vector.memset(ssb, 0.0)
                nc.vector.tensor_scalar_mul(
                    ssb[0:64, 0:64], atps[0:64, :], gamma_sb[0:64]
                )
                nc.vector.tensor_scalar_mul(
                    ssb[64:128, 64:128], atps[64:128, :], gamma_sb[64:128]
                )
                nc.vector.tensor_add(
                    ssb[0:64, 0:64], ssb[0:64, 0:64], ident_sb[0:64, :]
                )
                nc.vector.tensor_add(
                    ssb[64:128, 64:128], ssb[64:128, 64:128], ident_sb[64:128, :]
                )
                s_hi = spool.tile([128, 128], bf16)
                nc.vector.tensor_copy(out=s_hi, in_=ssb)
                s_lo = spool.tile([128, 128], bf16)
                nc.vector.tensor_tensor(
                    out=s_lo, in0=ssb, in1=s_hi, op=mybir.AluOpType.subtract
                )

                # ---- Phase 2: out = S_hi@hi + S_hi@lo + S_lo@hi ----
                for t in range(ntiles):
                    hi, lo = hitiles[t], lotiles[t]
                    osb = opool.tile([128, tile_f], fp32)
                    for s in range(slabs_per_tile):
                        sl = slice(s * SLAB, (s + 1) * SLAB)
                        ops = ops_pool.tile([128, SLAB], fp32)
                        nc.tensor.matmul(
                            ops, lhsT=s_hi, rhs=hi[:, sl], start=True, stop=False
                        )
                        nc.tensor.matmul(
                            ops,
                            lhsT=s_hi,
                            rhs=lo[:, sl],
                            start=False,
                            stop=not USE_LO_PHASE2,
                        )
                        if USE_LO_PHASE2:
                            nc.tensor.matmul(
                                ops,
                                lhsT=s_lo,
                                rhs=hi[:, sl],
                                start=False,
                                stop=True,
                            )
                        # Alternate DVE/ACT so phase-2 copies don't queue
                        # behind the next batch's phase-1 copies on DVE.
                        if s % 2 == 0:
                            nc.vector.tensor_copy(out=osb[:, sl], in_=ops)
                        else:
                            nc.scalar.copy(out=osb[:, sl], in_=ops)
                    nc.scalar.dma_start(
                        out=out[b, :, t * tile_f : (t + 1) * tile_f],
                        in_=osb[0:64, :],
                    )
                    nc.scalar.dma_start(
                        out=out[b, :, half + t * tile_f : half + (t + 1) * tile_f],
                        in_=osb[64:128, :],
                    )

    if not nc.is_finalized():
        nc.finalize()
    return nc


def _make_ident():
    ident = np.zeros((128, 64), np.float32)
    ident[np.arange(64), np.arange(64)] = 1.0
    ident[64 + np.arange(64), np.arange(64)] = 1.0
    return ident


def _make_identb():
    import ml_dtypes

    return np.eye(128, dtype=ml_dtypes.bfloat16)


def _setup_trace_hook():
    """Register the axon NTFF profiling hook (the image's antenv lacks the
    axon_hooks shim module; rebuild it and wire it to libaxon_pjrt.so)."""
    import sys
    import types

    import antenv

    if "antenv.axon_hooks" not in sys.modules:
        mod = types.ModuleType("antenv.axon_hooks")
        mod._hook = None

        def set_axon_ntff_profile_hook(h):
            mod._hook = h

        def get_axon_ntff_profile_hook():
            return mod._hook

        mod.set_axon_ntff_profile_hook = set_axon_ntff_profile_hook
        mod.get_axon_ntff_profile_hook = get_axon_ntff_profile_hook
        sys.modules["antenv.axon_hooks"] = mod
        antenv.axon_hooks = mod

    hooks = sys.modules["antenv.axon_hooks"]
    if hooks.get_axon_ntff_profile_hook() is None:
        from trn_agent_boot.trn_boot import _ntff_profile_via_ctypes

        hooks.set_axon_ntff_profile_hook(
            _ntff_profile_via_ctypes("/opt/axon/libaxon_pjrt.so")
        )

    # No S3 in this container: keep profile artifacts local.
    import concourse.bass_utils as bu

    bu.upload_artifacts = lambda tmpdir: tmpdir


def run(x, gamma, trace=False, tmpdir=None):
    """Run the SPMD kernel on 8 cores. Returns (out, exec_time_ns_or_None)."""
    from concourse.bass_utils import run_bass_kernel_spmd

    if trace:
        try:
            _setup_trace_hook()
        except Exception as e:  # tracing is best-effort; execution still works
            print("trace setup failed:", e)

    x = np.ascontiguousarray(np.asarray(x, dtype=np.float32))
    gamma = np.ascontiguousarray(np.asarray(gamma, dtype=np.float32))
    assert x.shape == (B, C, H, W), x.shape

    nc = build_cam_program()
    ident = _make_ident()
    identb = _make_identb()
    xr = x.reshape(NCORES, BPC, C, N)
    in_maps = [
        {
            "x": np.ascontiguousarray(xr[i]),
            "gamma": gamma,
            "ident": ident,
            "identb": identb,
        }
        for i in range(NCORES)
    ]
    res = run_bass_kernel_spmd(
        nc, in_maps, core_ids=list(range(NCORES)), trace=trace, tmpdir=tmpdir
    )
    outs = np.stack([np.asarray(res.results[i]["out"]) for i in range(NCORES)])
    y = outs.reshape(B, C, H, W).astype(np.float32)
    return y, res.exec_time_ns


def kernel(x, gamma):
    y, _ = run(x, gamma)
    return y
